# revision 13
# baseline (speedup 1.0000x reference)
"""ChannelWiseFC2d Trainium2 kernel (8 NeuronCores, channel-parallel).

Per (n, c): sort the 1024-vector x[n, c] descending, then
y[n, c, o] = sigmoid(sum_x sorted[x] * W[c, o, x] + b[c, o]).

Sharding: channels 64 -> 8 per core (pure expert parallelism, no
collectives). Per core:
  - bf16 bitonic sort (55 stages) of 2048 rows x 1024 on the DVE.
    Layout trick: the row-block dim t is INNERMOST in SBUF (element i
    of row t at free offset i*TG + t), so every compare-exchange pass
    streams contiguous runs of TG*d elements -- avoiding the ~1.3
    cycle/run AP-step penalty that makes small-d stages 2-2.5x slow
    in the natural layout. Host supplies x pre-interleaved.
  - Two UNEVEN groups (12 + 4 row-blocks): the big group's GEMM
    overlaps the small group's sort, so only the small group's GEMM
    remains as the serial tail.
  - PE transposes sorted 128x128 tiles (x onto partitions) -> lhsT.
  - bf16 matmul vs host-pretransposed W^T tiles, fp32 PSUM accum;
    bias via a K=1 matmul of ones^T @ b; sigmoid on ACT; DMA out.
Host pre/post: x,W,b cast to bf16, W transposed to [c, x, o],
output gathered and transposed to (256, 64, 1024) f32.
"""

import sys

sys.path.insert(0, "/opt/trn_rl_repo")

import numpy as np
import ml_dtypes

import concourse.bass as bass  # noqa: F401  (registers engine classes)
import concourse.mybir as mybir
from concourse import bacc
from concourse.tile import TileContext
from concourse.masks import make_identity
from concourse.bass_utils import run_bass_kernel_spmd

N, C, HW, OUT = 256, 64, 1024, 1024
N_CORES = 8
C_PER = C // N_CORES          # 8 channels per core
ROWS = C_PER * N              # 2048 rows of 1024 per core
NT = ROWS // 128              # 16 row-blocks of 128
GROUP_T = [12, 4]             # row-blocks per group (channel-aligned, uneven)
BF16 = mybir.dt.bfloat16
F32 = mybir.dt.float32
MAX_OP = mybir.AluOpType.max
MIN_OP = mybir.AluOpType.min


def _emit_sort(nc, zbufs, tg, n=HW, k_lo=2, k_hi=HW, cur=0):
    """Bitonic descending sort (levels k_lo..k_hi) of each row of
    zbufs[cur] ([128, n, tg] bf16, row t's element i at free position
    i*tg+t), ping-ponging between zbufs[0]/zbufs[1]. Returns the index
    of the buffer holding the result."""
    k = k_lo
    while k <= k_hi:
        d = k // 2
        while d >= 1:
            src, dst = zbufs[cur], zbufs[1 - cur]
            if k < n:
                a, bsub = n // (2 * k), k // (2 * d)
                pat = "p (a two bsub half d) t -> p two half a bsub (d t)"
                vs = src.rearrange(pat, a=a, two=2, bsub=bsub, half=2, d=d)
                vd = dst.rearrange(pat, a=a, two=2, bsub=bsub, half=2, d=d)
                for two in (0, 1):
                    desc = two == 0
                    nc.vector.tensor_tensor(
                        out=vd[:, two, 0], in0=vs[:, two, 0], in1=vs[:, two, 1],
                        op=MAX_OP if desc else MIN_OP)
                    nc.vector.tensor_tensor(
                        out=vd[:, two, 1], in0=vs[:, two, 0], in1=vs[:, two, 1],
                        op=MIN_OP if desc else MAX_OP)
            else:
                bsub = n // (2 * d)
                pat = "p (bsub half d) t -> p half bsub (d t)"
                vs = src.rearrange(pat, bsub=bsub, half=2, d=d)
                vd = dst.rearrange(pat, bsub=bsub, half=2, d=d)
                nc.vector.tensor_tensor(
                    out=vd[:, 0], in0=vs[:, 0], in1=vs[:, 1], op=MAX_OP)
                nc.vector.tensor_tensor(
                    out=vd[:, 1], in0=vs[:, 0], in1=vs[:, 1], op=MIN_OP)
            cur = 1 - cur
            d //= 2
        k *= 2
    return cur


def _build():
    nc = bacc.Bacc("TRN2", target_bir_lowering=False, debug=False,
                   num_devices=N_CORES)
    # x is one flat [128, HW * NT] fp16 image per partition; group g's
    # block starts at element offset sum(GROUP_T[:g]) * HW and holds
    # [HW, tg] t-innermost data.
    x_ext = nc.declare_dram_parameter("x", [128, HW * NT], BF16, isOutput=False)
    wt_ext = nc.declare_dram_parameter("wt", [C_PER, HW, OUT], BF16,
                                       isOutput=False)
    b_ext = nc.declare_dram_parameter("b", [C_PER, OUT], BF16, isOutput=False)
    out_ext = nc.declare_dram_parameter("out", [C_PER, N, OUT], F32,
                                        isOutput=True)

    w_v = wt_ext.ap().rearrange("c (k p) o -> p c k o", p=128)  # [128, 8, 8, 1024]

    with TileContext(nc) as tc:
        with (
            tc.tile_pool(name="consts", bufs=1) as cpool,
            tc.tile_pool(name="z", bufs=1) as zpool,
            tc.tile_pool(name="st", bufs=1) as stpool,
            tc.tile_pool(name="w", bufs=3) as wpool,
            tc.tile_pool(name="osb", bufs=4) as opool,
            tc.tile_pool(name="tp_psum", bufs=4, space="PSUM") as tppool,
            tc.tile_pool(name="mm_psum", bufs=4, space="PSUM") as mmpool,
        ):
            identity = cpool.tile([128, 128], BF16, tag="ident")
            make_identity(nc, identity)
            ones = cpool.tile([1, 128], BF16, tag="ones")
            nc.gpsimd.memset(ones, 1.0)
            b_sb = cpool.tile([1, C_PER, OUT], BF16, tag="bias")
            nc.sync.dma_start(out=b_sb, in_=b_ext.ap().unsqueeze(0))

            def emit_gemm(g, tg, t_off, zs, copy_engines):
                # Transpose sorted tiles (x onto partitions), then per-channel
                # GEMM + bias + sigmoid + store.
                st = stpool.tile([128, tg, HW // 128, 128], BF16, tag=f"st{g}")
                for i, (t, k) in enumerate(
                        (t, k) for t in range(tg) for k in range(HW // 128)):
                    ps = tppool.tile([128, 128], BF16, tag="tp")
                    nc.tensor.transpose(
                        ps, zs[:, k * 128:(k + 1) * 128, t], identity)
                    copy_engines[i % len(copy_engines)](st[:, t, k, :], ps)
                for cl in range(tg // 2):
                    c = t_off // 2 + cl
                    w_sb = wpool.tile([128, HW // 128, OUT], BF16, tag="w")
                    nc.sync.dma_start(out=w_sb, in_=w_v[:, c])
                    for nt in range(2):
                        t = cl * 2 + nt
                        for oh in range(2):
                            psum = mmpool.tile([128, 512], F32, tag="mm")
                            for k in range(HW // 128):
                                nc.tensor.matmul(
                                    psum,
                                    lhsT=st[:, t, k, :],
                                    rhs=w_sb[:, k, oh * 512:(oh + 1) * 512],
                                    start=(k == 0), stop=False)
                            nc.tensor.matmul(
                                psum, lhsT=ones,
                                rhs=b_sb[:, c, oh * 512:(oh + 1) * 512],
                                start=False, stop=True)
                            o_sb = opool.tile([128, 512], F32, tag="o")
                            nc.scalar.activation(
                                o_sb, psum, mybir.ActivationFunctionType.Sigmoid)
                            nc.sync.dma_start(
                                out=out_ext.ap()[c, nt * 128:(nt + 1) * 128,
                                                 oh * 512:(oh + 1) * 512],
                                in_=o_sb)

            tg0, tg1 = GROUP_T
            zb = []
            for g, tg in enumerate(GROUP_T):
                zb.append([zpool.tile([128, HW, tg], BF16, tag=f"z0g{g}",
                                      name=f"z0g{g}"),
                           zpool.tile([128, HW, tg], BF16, tag=f"z1g{g}",
                                      name=f"z1g{g}")])
            nc.sync.dma_start(
                out=zb[0][0].rearrange("p i t -> p (i t)"),
                in_=x_ext.ap()[:, 0:tg0 * HW])
            nc.sync.dma_start(
                out=zb[1][0].rearrange("p i t -> p (i t)"),
                in_=x_ext.ap()[:, tg0 * HW:NT * HW])
            act_copy = lambda o, i: nc.scalar.copy(o, i)  # noqa: E731
            dve_copy = lambda o, i: nc.vector.tensor_copy(o, i)  # noqa: E731
            cur0 = _emit_sort(nc, zb[0], tg0)
            emit_gemm(0, tg0, 0, zb[0][cur0], [act_copy])
            cur1 = _emit_sort(nc, zb[1], tg1)
            emit_gemm(1, tg1, tg0, zb[1][cur1], [act_copy, dve_copy])
    nc.finalize()
    return nc


_NC = None


def _get_nc():
    global _NC
    if _NC is None:
        _NC = _build()
    return _NC


def kernel(x, W, b):
    x = np.asarray(x)
    W = np.asarray(W)
    b = np.asarray(b)
    xt = x.reshape(N, C, HW).transpose(1, 0, 2)                  # (64, 256, 1024)
    x_f16 = xt.astype(ml_dtypes.bfloat16)
    wt_f16 = W.transpose(0, 2, 1).astype(ml_dtypes.bfloat16)             # (64, x, o)
    b_f16 = b.astype(ml_dtypes.bfloat16)
    in_maps = []
    for m in range(N_CORES):
        xc = x_f16[m * C_PER:(m + 1) * C_PER].reshape(NT, 128, HW)
        # per group: [128, HW, tg] t-innermost, then concat along free dim
        parts = []
        t_off = 0
        for tg in GROUP_T:
            blk = xc[t_off:t_off + tg]                 # [tg, 128, HW]
            parts.append(blk.transpose(1, 2, 0).reshape(128, HW * tg))
            t_off += tg
        in_maps.append({
            "x": np.ascontiguousarray(np.concatenate(parts, axis=1)),
            "wt": np.ascontiguousarray(wt_f16[m * C_PER:(m + 1) * C_PER]),
            "b": np.ascontiguousarray(b_f16[m * C_PER:(m + 1) * C_PER]),
        })
    res = run_bass_kernel_spmd(_get_nc(), in_maps, core_ids=list(range(N_CORES)))
    out = np.concatenate([res.results[m]["out"] for m in range(N_CORES)], axis=0)
    return np.ascontiguousarray(out.transpose(1, 0, 2)).astype(np.float32)
